# revision 2
# baseline (speedup 1.0000x reference)
"""Distributed Trainium2 kernel for AttHGCNConv:
out = LeakyReLU_0.2( A @ B @ (B.T @ (A.T @ embs)) ),  A=att_adj [N,E], B=inp_adj [E,N].

Never materializes adj = A@B; chains 4 thin matmuls of 34 GFLOP each.
8-way sharded, ALL-GATHER ONLY dataflow, 4-way staircase pipeline:

Each stage computes its [1024, 256] output shard in 4 m-parts of 2 m-tiles;
part i is computed front-to-back (all 64 k-tiles) so its AllGather fires at
~25*(i+1)% of the stage instead of at the end.  The 4 small AGs (128KB in /
1MB out, fp16 wire) pipeline back-to-back on the CC stream and land while the
producer is still computing; the consumer k-consumes gathered rank-blocks in
arrival order (host-side permutation of the lhsT k-tile layout), so only the
last AG's tail is ever exposed (~1-2us/stage).

Queues: weights on sync, embs preload + evictions on scalar, gathered-block
reloads interleaved between AG emissions on the gpsimd stream (so each reload
waits only on an AG that already completed).  A tiny warm-up AllGather
absorbs cross-core launch skew before the first real AG.
"""

import sys

for p in ("/opt/trn_rl_repo", "/root/.axon_site"):
    if p not in sys.path:
        sys.path.insert(0, p)

import numpy as np

import concourse.bass as bass  # noqa: F401
import concourse.mybir as mybir
import concourse.tile as tile
from concourse import bacc
from concourse.bass_utils import run_bass_kernel_spmd

N_CORES = 8
N = 8192  # nodes
E = 8192  # hyperedges
D = 256   # embedding dim
S = E // N_CORES   # 1024 per-core shard
KT = 128           # partition tile
NK = N // KT       # 64 k-tiles
SK = S // KT       # 8 m-tiles
LEAKY = 0.2

NP = 4             # m-parts per stage (staircase granularity)
MT = SK // NP      # 2 m-tiles per part
KP = 4             # k-parts (match: consumer chunk = producer part)
KPT = NK // KP     # 16 k-tiles per k-part
BW = 4             # k-tiles fused per weight DMA row
GR = KPT // BW     # 4 weight rows per (part, k-part)
MW = MT * KT       # 256 m-cols per part
ROWW = BW * MW     # 1024 elements per weight row
NROWS = NP * KP * GR  # 64 rows per stage

W16 = mybir.dt.float16
F32 = mybir.dt.float32
NP16 = np.float16

_CACHED_NC = None


def _build():
    nc = bacc.Bacc("TRN2", target_bir_lowering=False, debug=False,
                   num_devices=N_CORES)

    wts = [nc.dram_tensor(f"w{s}", [NROWS, KT, ROWW], W16,
                          kind="ExternalInput") for s in range(4)]
    e_g = nc.dram_tensor("e_g", [KP, KT, KPT * D], W16, kind="ExternalInput")
    out = nc.dram_tensor("out", [S, D], F32, kind="ExternalOutput")

    out_v = out.ap().rearrange("(k p) d -> p k d", p=KT)
    rg = [list(range(N_CORES))]

    with tile.TileContext(nc) as tc:
        with (
            tc.tile_pool(name="w", bufs=12) as wpool,
            tc.tile_pool(name="e", bufs=4) as epool,
            tc.tile_pool(name="g", bufs=16) as gpool,
            tc.tile_pool(name="keep", bufs=1) as keep,
            tc.tile_pool(name="ps", bufs=3, space="PSUM") as pspool,
            tc.tile_pool(name="dram", bufs=1, space="DRAM") as dram,
        ):
            # per (stage-boundary, part) bounce buffers; partition-major so
            # rank r's block in the gathered output is rows [128r,128r+128)
            cc_ins = [[dram.tile([KT, MT * D], W16,
                                 name=f"cci_{b}_{i}", tag=f"cci{b}{i}")
                       for i in range(NP)] for b in range(3)]
            cc_outs = [[dram.tile([KT * N_CORES, MT * D], W16,
                                  addr_space="Shared", name=f"cco_{b}_{i}",
                                  tag=f"cco{b}{i}") for i in range(NP)]
                       for b in range(3)]

            # tiny skew-absorber collective: syncs the cc stream across the
            # 8 cores during S1's compute so the first real AG doesn't pay
            # the cross-core launch-skew wait.
            warm_in = dram.tile([KT, 8], W16, name="warm_in", tag="wi")
            warm_out = dram.tile([KT * N_CORES, 8], W16, addr_space="Shared",
                                 name="warm_out", tag="wo")
            nc.gpsimd.collective_compute(
                "AllGather", mybir.AluOpType.bypass, replica_groups=rg,
                ins=[warm_in[:].opt()], outs=[warm_out[:].opt()])

            # ---- embs preload: 4 x 1MB on the (otherwise idle) scalar
            # queue so stage-1 weight rows flow on sync from t=0 ----
            es = []
            for ge in range(KP):
                er = epool.tile([KT, KPT * D], W16, name="er", tag="e")
                nc.scalar.dma_start(er[:], e_g.ap()[ge])
                es.append(er)

            # gathered-rank-block reload machinery.  blocks[b][j][r] is the
            # [128, MT*D] SBUF copy of rank r's part-j contribution at
            # boundary b.  Reload DMAs ride the gpsimd stream, emitted right
            # after AG_{j+1} of the same boundary (so the wait they carry on
            # AG_j is already satisfied and never blocks the stream).
            blocks = [[[None] * N_CORES for _ in range(KP)] for _ in range(3)]

            def emit_reloads(b, j):
                if not (0 <= j < KP):
                    return
                for r in range(N_CORES):
                    bt = gpool.tile([KT, MT * D], W16, name="gr", tag=f"g{j}")
                    nc.gpsimd.dma_start(
                        bt[:], cc_outs[b][j][r * KT:(r + 1) * KT, :])
                    blocks[b][j][r] = bt

            def gathered_rhs(b):
                def rhs(t):
                    j, rem = divmod(t, KPT)
                    r, u = divmod(rem, MT)
                    return blocks[b][j][r][:, u * D:(u + 1) * D]
                return rhs

            def stage(w_t, rhs_of, sink, on_part=None):
                """4-part staircase: part i computed over all 64 k-tiles
                (k-parts in gathered-arrival order), then sink(i, ps) evicts
                and on_part(i) interleaves next-boundary reloads."""
                for i in range(NP):
                    ps = [pspool.tile([KT, D], F32, name=f"ps{u}",
                                      tag=f"ps{u}") for u in range(MT)]
                    for j in range(KP):
                        for g in range(GR):
                            row = (i * KP + j) * GR + g
                            w = wpool.tile([KT, ROWW], W16, name="w", tag="w")
                            nc.sync.dma_start(w[:], w_t.ap()[row])
                            for kk in range(BW):
                                t = j * KPT + g * BW + kk
                                rh = rhs_of(t)
                                for u in range(MT):
                                    nc.tensor.matmul(
                                        ps[u][:],
                                        w[:, kk * MW + u * KT:
                                          kk * MW + (u + 1) * KT],
                                        rh, start=(t == 0), stop=(t == NK - 1))
                    sink(i, ps)
                    if on_part is not None:
                        on_part(i)

            def ag_sink(b, t_sb):
                def sink(i, ps):
                    for u in range(MT):
                        dst = t_sb[:, (i * MT + u) * D:(i * MT + u + 1) * D]
                        if u == 0:
                            nc.vector.tensor_copy(dst, ps[u][:])
                        else:
                            nc.scalar.copy(dst, ps[u][:])
                    nc.scalar.dma_start(
                        cc_ins[b][i][:],
                        t_sb[:, i * MT * D:(i + 1) * MT * D])
                    nc.gpsimd.collective_compute(
                        "AllGather", mybir.AluOpType.bypass,
                        replica_groups=rg,
                        ins=[cc_ins[b][i][:].opt()],
                        outs=[cc_outs[b][i][:].opt()])
                return sink

            # ---- S1: t1 = A[:,e_c].T @ embs ----
            t1 = keep.tile([KT, SK * D], W16, name="t1", tag="t1")
            stage(wts[0],
                  lambda t: es[t // KPT][:, (t % KPT) * D:(t % KPT + 1) * D],
                  ag_sink(0, t1),
                  on_part=lambda i: emit_reloads(0, i - 1))

            # ---- S2: t2[n_c] = B[:,n_c].T @ t1_full ----
            emit_reloads(0, KP - 1)
            t2 = keep.tile([KT, SK * D], W16, name="t2", tag="t2")
            stage(wts[1], gathered_rhs(0), ag_sink(1, t2),
                  on_part=lambda i: emit_reloads(1, i - 1))

            # ---- S3: t3 = B[e_c,:] @ t2_full ----
            emit_reloads(1, KP - 1)
            t3 = keep.tile([KT, SK * D], W16, name="t3", tag="t3")
            stage(wts[2], gathered_rhs(1), ag_sink(2, t3),
                  on_part=lambda i: emit_reloads(2, i - 1))

            # ---- S4: out[n_c] = A[n_c,:] @ t3_full, LeakyReLU fused ----
            emit_reloads(2, KP - 1)
            o = keep.tile([KT, SK * D], F32, name="o", tag="o")
            negs = [keep.tile([KT, D], F32, name=f"neg{u}", tag=f"neg{u}")
                    for u in range(MT)]

            def leaky_sink(i, ps):
                for u in range(MT):
                    gm = i * MT + u
                    nc.vector.tensor_scalar_mul(negs[u][:], ps[u][:], LEAKY)
                    nc.vector.tensor_max(
                        o[:, gm * D:(gm + 1) * D], ps[u][:], negs[u][:])
                    # stream each m-tile out as soon as its LeakyReLU lands
                    nc.sync.dma_start(
                        out_v[:, gm, :], o[:, gm * D:(gm + 1) * D])

            stage(wts[3], gathered_rhs(2), leaky_sink)

    nc.compile()
    return nc


# consumption order for gathered rhs: t = j*KPT + 2r + u  ->  k_global =
# r*SK + i*MT + u where the producing part i == consuming k-part j
_PERM_G = np.array([r * SK + j * MT + u
                    for j in range(KP) for r in range(N_CORES)
                    for u in range(MT)])
_PERM_ID = np.arange(NK)


def _relay(w, perm):
    """lhsT [8192, 1024] (k-rows, m-cols) -> [NROWS, KT, ROWW], k-tiles in
    consumption order `perm`, rows ordered (part, k-part, group)."""
    wt = w.reshape(NK, KT, S)[perm]             # [64, 128, 1024]
    wt = wt.reshape(NK, KT, NP, MW)
    wt = wt.transpose(2, 0, 1, 3)               # [NP, 64, 128, 256]
    wt = wt.reshape(NP, KP, GR, BW, KT, MW)
    wt = wt.transpose(0, 1, 2, 4, 3, 5)         # [NP, KP, GR, 128, BW, 256]
    return np.ascontiguousarray(wt).reshape(NROWS, KT, ROWW)


def _fuse_e(eb):
    # [N, D] -> [KP, 128, KPT*D]
    return np.ascontiguousarray(
        eb.reshape(KP, KPT, KT, D).transpose(0, 2, 1, 3)
    ).reshape(KP, KT, KPT * D)


def _shard_inputs(inp_adj, att_adj, embs):
    A = np.asarray(att_adj, dtype=np.float32)   # [N, E]
    B = np.asarray(inp_adj, dtype=np.float32)   # [E, N]
    eb = np.asarray(embs, dtype=np.float32).astype(NP16)   # [N, D]
    e_gh = _fuse_e(eb)
    in_maps = []
    for c in range(N_CORES):
        s = slice(c * S, (c + 1) * S)
        m = {
            "e_g": e_gh,
            "w0": _relay(A[:, s].astype(NP16), _PERM_ID),
            "w1": _relay(B[:, s].astype(NP16), _PERM_G),
            "w2": _relay(np.ascontiguousarray(B[s, :].T).astype(NP16),
                         _PERM_G),
            "w3": _relay(np.ascontiguousarray(A[s, :].T).astype(NP16),
                         _PERM_G),
        }
        in_maps.append(m)
    return in_maps


def _reset_device():
    """Recover wedged NeuronCores (NRT_EXEC_UNIT_UNRECOVERABLE) via axon."""
    import ctypes

    import jax
    try:
        jax.devices()
        lib = ctypes.CDLL("/opt/axon/libaxon_pjrt.so")
        lib.axon_reset.restype = ctypes.c_int64
        lib.axon_reset()
    except Exception:
        pass


def kernel(inp_adj, att_adj, embs, _trace=False):
    global _CACHED_NC
    if _CACHED_NC is None:
        _CACHED_NC = _build()
    nc = _CACHED_NC
    in_maps = _shard_inputs(inp_adj, att_adj, embs)
    try:
        res = run_bass_kernel_spmd(nc, in_maps,
                                   core_ids=list(range(N_CORES)),
                                   trace=_trace)
    except Exception:
        _reset_device()
        res = run_bass_kernel_spmd(nc, in_maps,
                                   core_ids=list(range(N_CORES)),
                                   trace=_trace)
    # core c owns out rows [c*S, (c+1)*S)
    full = np.empty((N, D), np.float32)
    for c in range(N_CORES):
        full[c * S:(c + 1) * S] = res.results[c]["out"]
    if _trace:
        kernel.last_exec_time_ns = res.exec_time_ns
    return full


# revision 3
# speedup vs baseline: 1.0625x; 1.0625x over previous
"""Distributed Trainium2 kernel for AttHGCNConv:
out = LeakyReLU_0.2( A @ B @ (B.T @ (A.T @ embs)) ),  A=att_adj [N,E], B=inp_adj [E,N].

Never materializes adj = A@B; chains 4 thin matmuls of 34 GFLOP each.
8-way sharded, ALL-GATHER ONLY dataflow, asymmetric 3-part staircase:

Each stage computes its [1024, 256] output shard in m-parts of (4, 3, 1)
m-tiles, each part front-to-back over all 64 k-tiles, so the part AllGathers
fire at 50% / 87.5% / 100% of the stage.  Measured AG cost on the single CC
stream is ~15-19us nearly independent of size (mesh fixed costs + HBM
contention), so only 3 AGs/boundary (~45us CC time < 66us stage) with the
big one earliest.  The consumer k-consumes gathered rank-blocks in arrival
order (host-side permutation of the lhsT k-tile layout); with (32, 24, 8)
k-chunks every chunk lands ~6us before first use in steady state.

Queues: weights on sync, embs preload + cc_in writes + gathered-block
reloads on scalar (each reload carries a wait on an AG that lands before
anything queued behind it is needed), AG triggers alone on gpsimd, evict
casts + LeakyReLU on vector.  fp16 operands (PSUM f32), fp16 wires.
"""

import sys

for p in ("/opt/trn_rl_repo", "/root/.axon_site"):
    if p not in sys.path:
        sys.path.insert(0, p)

import numpy as np

import concourse.bass as bass  # noqa: F401
import concourse.mybir as mybir
import concourse.tile as tile
from concourse import bacc
from concourse.bass_utils import run_bass_kernel_spmd

N_CORES = 8
N = 8192  # nodes
E = 8192  # hyperedges
D = 256   # embedding dim
S = E // N_CORES   # 1024 per-core shard
KT = 128           # partition tile
NK = N // KT       # 64 k-tiles
SK = S // KT       # 8 m-tiles
LEAKY = 0.2

PARTS = ((0, 4), (4, 3), (7, 1))   # (m-tile start, count) per staircase part
KOFF = (0, 32, 56)                 # consumer k-chunk offsets (k-tiles)
KCNT = (32, 24, 8)                 # consumer k-chunk sizes = 8 * mcnt
BW = 4                             # k-tiles fused per weight DMA row
NROWS = NK // BW                   # 16 weight rows per (stage, part)
EB = 16                            # embs k-tiles per DMA chunk

W16 = mybir.dt.float16
F32 = mybir.dt.float32
NP16 = np.float16

_CACHED_NC = None


def _build():
    nc = bacc.Bacc("TRN2", target_bir_lowering=False, debug=False,
                   num_devices=N_CORES)

    wts = [[nc.dram_tensor(f"w{s}p{i}", [NROWS, KT, BW * mc * KT], W16,
                           kind="ExternalInput")
            for i, (ms, mc) in enumerate(PARTS)] for s in range(4)]
    e_g = nc.dram_tensor("e_g", [NK // EB, KT, EB * D], W16,
                         kind="ExternalInput")
    out = nc.dram_tensor("out", [S, D], F32, kind="ExternalOutput")

    out_v = out.ap().rearrange("(k p) d -> p k d", p=KT)
    rg = [list(range(N_CORES))]

    with tile.TileContext(nc) as tc:
        with (
            tc.tile_pool(name="w", bufs=6) as wpool,
            tc.tile_pool(name="e", bufs=4) as epool,
            tc.tile_pool(name="g", bufs=16) as gpool,
            tc.tile_pool(name="keep", bufs=1) as keep,
            tc.tile_pool(name="ps", bufs=8, space="PSUM") as pspool,
            tc.tile_pool(name="dram", bufs=1, space="DRAM") as dram,
        ):
            # per (stage-boundary, part) bounce buffers; partition-major so
            # rank r's block in the gathered output is rows [128r,128r+128)
            cc_ins = [[dram.tile([KT, mc * D], W16,
                                 name=f"cci_{b}_{i}", tag=f"cci{b}{i}")
                       for i, (ms, mc) in enumerate(PARTS)] for b in range(3)]
            cc_outs = [[dram.tile([KT * N_CORES, mc * D], W16,
                                  addr_space="Shared", name=f"cco_{b}_{i}",
                                  tag=f"cco{b}{i}")
                        for i, (ms, mc) in enumerate(PARTS)] for b in range(3)]

            # ---- embs preload: 4 x 1MB on the (otherwise idle) scalar
            # queue so stage-1 weight rows flow on sync from t=0 ----
            es = []
            for ge in range(NK // EB):
                er = epool.tile([KT, EB * D], W16, name="er", tag="e")
                nc.scalar.dma_start(er[:], e_g.ap()[ge])
                es.append(er)

            # gathered-rank-block reloads ride the scalar queue, emitted so
            # that each one's wait (on its AG) is satisfied before anything
            # queued behind it is needed.
            blocks = [[[None] * N_CORES for _ in range(3)] for _ in range(3)]

            def emit_reloads(b, j):
                if not (0 <= j < 3):
                    return
                mc = PARTS[j][1]
                for r in range(N_CORES):
                    bt = gpool.tile([KT, mc * D], W16, name="gr", tag=f"g{j}")
                    nc.scalar.dma_start(
                        bt[:], cc_outs[b][j][r * KT:(r + 1) * KT, :])
                    blocks[b][j][r] = bt

            def gathered_rhs(b):
                def rhs(t):
                    j = 2 if t >= KOFF[2] else (1 if t >= KOFF[1] else 0)
                    r, u = divmod(t - KOFF[j], PARTS[j][1])
                    return blocks[b][j][r][:, u * D:(u + 1) * D]
                return rhs

            def stage(w_list, rhs_of, sink, on_part=None):
                """Asymmetric staircase: part i (mc m-tiles) computed over
                all 64 k-tiles (arrival order), then sink evicts + AGs."""
                for pi, (ms, mc) in enumerate(PARTS):
                    ps = [pspool.tile([KT, D], F32, name=f"ps{u}", tag="ps")
                          for u in range(mc)]
                    for g in range(NROWS):
                        w = wpool.tile([KT, BW * mc * KT], W16, name="w",
                                       tag=f"w{pi}")
                        nc.sync.dma_start(w[:], w_list[pi].ap()[g])
                        for kk in range(BW):
                            t = g * BW + kk
                            rh = rhs_of(t)
                            for u in range(mc):
                                nc.tensor.matmul(
                                    ps[u][:],
                                    w[:, kk * mc * KT + u * KT:
                                      kk * mc * KT + (u + 1) * KT],
                                    rh, start=(t == 0), stop=(t == NK - 1))
                    sink(pi, ms, mc, ps)
                    if on_part is not None:
                        on_part(pi)

            def ag_sink(b, t_sb):
                def sink(pi, ms, mc, ps):
                    for u in range(mc):
                        nc.vector.tensor_copy(
                            t_sb[:, (ms + u) * D:(ms + u + 1) * D], ps[u][:])
                    nc.scalar.dma_start(
                        cc_ins[b][pi][:], t_sb[:, ms * D:(ms + mc) * D])
                    nc.gpsimd.collective_compute(
                        "AllGather", mybir.AluOpType.bypass,
                        replica_groups=rg,
                        ins=[cc_ins[b][pi][:].opt()],
                        outs=[cc_outs[b][pi][:].opt()])
                return sink

            # ---- S1: t1 = A[:,e_c].T @ embs ----
            t1 = keep.tile([KT, SK * D], W16, name="t1", tag="t1")
            stage(wts[0],
                  lambda t: es[t // EB][:, (t % EB) * D:(t % EB + 1) * D],
                  ag_sink(0, t1),
                  on_part=lambda pi: emit_reloads(0, pi - 1))

            # ---- S2: t2[n_c] = B[:,n_c].T @ t1_full ----
            emit_reloads(0, 2)
            t2 = keep.tile([KT, SK * D], W16, name="t2", tag="t2")
            stage(wts[1], gathered_rhs(0), ag_sink(1, t2),
                  on_part=lambda pi: emit_reloads(1, pi - 1))

            # ---- S3: t3 = B[e_c,:] @ t2_full ----
            emit_reloads(1, 2)
            t3 = keep.tile([KT, SK * D], W16, name="t3", tag="t3")
            stage(wts[2], gathered_rhs(1), ag_sink(2, t3),
                  on_part=lambda pi: emit_reloads(2, pi - 1))

            # ---- S4: out[n_c] = A[n_c,:] @ t3_full, LeakyReLU fused ----
            emit_reloads(2, 2)
            o = keep.tile([KT, SK * D], F32, name="o", tag="o")
            neg = keep.tile([KT, D], F32, name="neg", tag="neg")

            def leaky_sink(pi, ms, mc, ps):
                for u in range(mc):
                    gm = ms + u
                    nc.vector.tensor_scalar_mul(neg[:], ps[u][:], LEAKY)
                    nc.vector.tensor_max(
                        o[:, gm * D:(gm + 1) * D], ps[u][:], neg[:])
                    # stream each m-tile out as soon as its LeakyReLU lands
                    nc.sync.dma_start(
                        out_v[:, gm, :], o[:, gm * D:(gm + 1) * D])

            stage(wts[3], gathered_rhs(2), leaky_sink)

    nc.compile()
    return nc


# consumption order for gathered rhs: within chunk j (= producer part j),
# blocks arrive rank-major: t = KOFF[j] + r*mc + u  ->  k_global =
# r*SK + PARTS[j][0] + u
_PERM_G = np.array([r * SK + PARTS[j][0] + u
                    for j in range(3) for r in range(N_CORES)
                    for u in range(PARTS[j][1])])
_PERM_ID = np.arange(NK)


def _relay(w, perm):
    """lhsT [8192, 1024] (k-rows, m-cols) -> per-part arrays
    [NROWS, KT, BW*mc*KT], k-tiles in consumption order `perm`."""
    wt = w.reshape(NK, KT, S)[perm]             # [64, 128, 1024]
    outs = []
    for ms, mc in PARTS:
        cols = wt[:, :, ms * KT:(ms + mc) * KT]  # [64, 128, mc*128]
        outs.append(np.ascontiguousarray(
            cols.reshape(NROWS, BW, KT, mc * KT).transpose(0, 2, 1, 3)
        ).reshape(NROWS, KT, BW * mc * KT))
    return outs


def _fuse_e(eb):
    # [N, D] -> [NK/EB, 128, EB*D]
    return np.ascontiguousarray(
        eb.reshape(NK // EB, EB, KT, D).transpose(0, 2, 1, 3)
    ).reshape(NK // EB, KT, EB * D)


def _shard_inputs(inp_adj, att_adj, embs):
    A = np.asarray(att_adj, dtype=np.float32)   # [N, E]
    B = np.asarray(inp_adj, dtype=np.float32)   # [E, N]
    eb = np.asarray(embs, dtype=np.float32).astype(NP16)   # [N, D]
    e_gh = _fuse_e(eb)
    in_maps = []
    for c in range(N_CORES):
        s = slice(c * S, (c + 1) * S)
        shards = {
            0: _relay(A[:, s].astype(NP16), _PERM_ID),
            1: _relay(B[:, s].astype(NP16), _PERM_G),
            2: _relay(np.ascontiguousarray(B[s, :].T).astype(NP16), _PERM_G),
            3: _relay(np.ascontiguousarray(A[s, :].T).astype(NP16), _PERM_G),
        }
        m = {"e_g": e_gh}
        for sname, ws in shards.items():
            for i, wa in enumerate(ws):
                m[f"w{sname}p{i}"] = wa
        in_maps.append(m)
    return in_maps


def _reset_device():
    """Recover wedged NeuronCores (NRT_EXEC_UNIT_UNRECOVERABLE) via axon."""
    import ctypes

    import jax
    try:
        jax.devices()
        lib = ctypes.CDLL("/opt/axon/libaxon_pjrt.so")
        lib.axon_reset.restype = ctypes.c_int64
        lib.axon_reset()
    except Exception:
        pass


def kernel(inp_adj, att_adj, embs, _trace=False):
    global _CACHED_NC
    if _CACHED_NC is None:
        _CACHED_NC = _build()
    nc = _CACHED_NC
    in_maps = _shard_inputs(inp_adj, att_adj, embs)
    try:
        res = run_bass_kernel_spmd(nc, in_maps,
                                   core_ids=list(range(N_CORES)),
                                   trace=_trace)
    except Exception:
        _reset_device()
        res = run_bass_kernel_spmd(nc, in_maps,
                                   core_ids=list(range(N_CORES)),
                                   trace=_trace)
    # core c owns out rows [c*S, (c+1)*S)
    full = np.empty((N, D), np.float32)
    for c in range(N_CORES):
        full[c * S:(c + 1) * S] = res.results[c]["out"]
    if _trace:
        kernel.last_exec_time_ns = res.exec_time_ns
    return full
